# revision 59
# baseline (speedup 1.0000x reference)
"""GCN layer (linear + weighted scatter-add aggregation) on 8 TRN2 NeuronCores.

Reference computation:
    h = x @ W.T                      [N, D]
    out[r] = sum_{e: row[e]==r} val[e] * h[col[e]]

Key identity: the linear layer commutes past the (linear) aggregation:
    out = (A @ x) @ W.T    where A[r,c] = sum of val over edges (r,c)
so we aggregate raw x rows first (8x less matmul work, no h materialization).

Distribution: destination nodes are sharded 12500/core (edges partitioned by
destination so the segment-sum is fully local; x is replicated to each core's
HBM by the host, so no collective is needed).

Per-core algorithm ("perm-pack", transposed-matmul variant):
  - Host packs destinations into "windows" of <=W_RANK=42 dest-items and
    4x256 edge slots, where the 4 quotas correspond to 4 source-node chunks
    of 25000 rows (dma_gather indices are int16). Dests may be split across
    windows when a quota or the rank cap binds; a rank cap of 42 (vs the
    32-item mean) keeps windows ~99% full. The packing order defines a
    per-core virtual destination numbering; output rows are inverse-permuted
    (and summed, if a dest was split) on the host.
  - A window is 8 "groups" of 128 edge slots (2 per chunk). One "call" = 4
    windows = 32 groups = 4096 slots: four GPSIMD dma_gather ucode calls
    (1024 int16 indices each, one per source chunk; the HW ucode rejects
    larger) pull the x rows (bf16, 512B each) into SBUF as
    [128 slots, 32 groups, 256]. A banded scaled one-hot S ([128, 42] per
    group: S[p,r] = val[p] * (rank[p]==r)) is built with 2 batched DVE ops
    per call. Gather indices and seg/val ride separate DRAM tensors on
    separate HWDGE queues (SP / Activation), software-pipelined 2 pairs /
    1 quad deep; idx loads touch only partitions 0..31 (exactly what the
    two queue-0 Q7 cpus read — each consumes its own 16-partition slice),
    and seg/val rows pack 4 pairs so the per-partition descriptor reaches
    512B (full DMA bandwidth in the cost model).
  - PE (transposed): the aggregate is accumulated as its TRANSPOSE
    aggT[feat, dest]: per group one matmul per 128-feature half with
    lhsT = gathered rows [128 slots, 128 feats] (stationary) and
    rhs = S band [128 slots, 42 ranks] (moving), accumulating the window's
    8 groups into PSUM [128 feats, 42 ranks]. Streaming 42 dest-columns
    instead of 256 feature-rows is >3x fewer PE cycles, and aggT lands
    already transposed for the W projection: outT[fo, d] =
    sum_fi W.T[fi, fo] * aggT[fi, d] via 4 accumulating matmuls. No PE
    transposes, no scatter, no atomics, no collectives. The host transposes
    the [feat, dest] output back at unpack time.
"""

import os
import sys

sys.path.insert(0, "/opt/trn_rl_repo")
os.environ.setdefault("MYCRO_LOCAL_CACHE", "1")

from contextlib import ExitStack

import numpy as np
import ml_dtypes

import concourse.bass as bass
import concourse.bacc as bacc
import concourse.mybir as mybir
import concourse.tile as tile
from concourse.bass_utils import run_bass_kernel_spmd
from concourse.library_config import mlp as _mlp_lib

N_NODES = 100000
N_CORES = 8
NPC = N_NODES // N_CORES  # dests per core
D = 256
SLOTS = 128  # edge slots per group (= matmul K)
W_RANK = 42  # dest-item rank cap per window (= matmul N in the transposed form)
NCHUNK = 4
CHUNK = 25000  # source rows per chunk (int16-addressable)
W_CQ = 256  # window chunk quota (2 groups per chunk)
W_GROUPS = 8  # groups per window
W_SLOTS = W_GROUPS * SLOTS  # 1024 edge slots per window
CG = 32  # groups per call (4 windows -> WPC*W_RANK virtual dests/call)
WPC = 4  # windows per call
VPC = WPC * W_RANK  # virtual dests per call
# One call per gather instruction: the HW gather ucode rejects num_idxs
# beyond 1024 (2048-idx gathers crash the NEFF despite passing CoreSim).
F_CALLS = 1
GATHER_IDX = F_CALLS * WPC * W_CQ  # 1024 indices per (pair, chunk) dma_gather

BF16 = ml_dtypes.bfloat16


# ----------------------------------------------------------------------------
# Host-side packing
# ----------------------------------------------------------------------------

def pack_core(rows_loc, cols, vals, npc):
    """Pack one core's edges (dest-local ids in [0, npc)) into windows.

    Dests are placed greedily (alternating big/small by total degree) and may
    be SPLIT across consecutive windows when a per-chunk quota or the rank
    cap is hit, so windows fill to ~100%. Split partial sums are re-combined
    on the host (np.add.at over duplicate dest ids).

    Returns per-slot arrays idx (in gather slot order) and seg/val
    (in per-call slot order), plus per-item vrow/dest.
    """
    chunk_id = cols // CHUNK
    key = rows_loc.astype(np.int64) * NCHUNK + chunk_id
    order = np.argsort(key, kind="stable")
    cols_s = cols[order]
    vals_s = vals[order]
    dc_deg = np.bincount(key, minlength=npc * NCHUNK).astype(np.int64)
    dc_deg = dc_deg.reshape(npc, NCHUNK)
    dc_start = np.zeros(npc * NCHUNK + 1, np.int64)
    dc_start[1:] = np.cumsum(dc_deg.ravel())
    dc_start = dc_start[:-1].reshape(npc, NCHUNK)
    deg = dc_deg.sum(1)

    # alternating big/small feed order balances window sums
    srt = np.argsort(deg, kind="stable")
    feed = np.empty(npc, np.int64)
    feed[0::2] = srt[::-1][: (npc + 1) // 2]
    feed[1::2] = srt[: npc // 2]

    # greedy fill with splitting: a carry queue keeps filling the current
    # window until its quotas (or the rank cap) are exhausted; unplaced
    # remainders of partially-placed dests go to the front of the queue
    from collections import deque

    queue = deque(
        (int(d), dc_deg[int(d)].copy(), np.zeros(NCHUNK, np.int64)) for d in feed
    )
    items_dest, items_w, items_rank = [], [], []
    items_take = []  # [NCHUNK] takes
    items_coff = []  # [NCHUNK] consumed offset within dest-chunk edges
    deferred = []  # items that fit nothing in the current window
    w = 0
    rank = 0
    rq = np.full(NCHUNK, W_CQ, np.int64)
    while queue or deferred:
        if not queue or rank == W_RANK:
            # close window; deferred items lead the next one
            for it in reversed(deferred):
                queue.appendleft(it)
            deferred = []
            w += 1
            rank = 0
            rq = np.full(NCHUNK, W_CQ, np.int64)
            continue
        d, rem, coff = queue.popleft()
        take = np.minimum(rem, rq)
        if take.sum() == 0:
            deferred.append((d, rem, coff))
            continue
        items_dest.append(d)
        items_w.append(w)
        items_rank.append(rank)
        items_take.append(take.copy())
        items_coff.append(coff.copy())
        rq -= take
        rank += 1
        rem = rem - take
        if rem.sum() > 0:
            deferred.append((d, rem, coff + take))
    n_windows = w + 1
    n_items = len(items_dest)
    items_dest = np.asarray(items_dest, np.int64)
    items_w = np.asarray(items_w, np.int64)
    items_rank = np.asarray(items_rank, np.int64)
    items_take = np.asarray(items_take, np.int64)  # [n_items, NCHUNK]
    items_coff = np.asarray(items_coff, np.int64)

    n_calls_local = (n_windows + WPC - 1) // WPC
    n_pairs = (n_calls_local + F_CALLS - 1) // F_CALLS
    n_calls_local = n_pairs * F_CALLS
    G = n_calls_local * CG

    # per-(item, chunk) slot bases in both slot orders
    call = items_w // WPC
    w_loc = items_w % WPC
    pair = call // F_CALLS
    q = call % F_CALLS
    # running offset within each window chunk quota
    qoff = np.zeros((n_items, NCHUNK), np.int64)
    cum = {}
    for i in range(n_items):
        ww = items_w[i]
        c0 = cum.get(ww)
        if c0 is None:
            c0 = np.zeros(NCHUNK, np.int64)
        qoff[i] = c0
        cum[ww] = c0 + items_take[i]
    # seg/val order: call*4096 + (8c + 2*w_loc)*128 + qoff
    base_sv = call * (CG * SLOTS)
    # idx order: pair*F*4096 + (8F*c + 8*q + 2*w_loc)*128 + qoff
    base_ix = pair * (F_CALLS * CG * SLOTS)

    flat_deg = items_take.ravel()
    cgrid = np.tile(np.arange(NCHUNK), n_items)
    irep = np.repeat(np.arange(n_items), NCHUNK)
    e_start = (dc_start[items_dest] + items_coff).ravel()
    sv_base = (
        base_sv[irep]
        + (8 * cgrid + 2 * w_loc[irep]) * SLOTS
        + qoff.ravel()
    )
    ix_base = (
        base_ix[irep]
        + (8 * F_CALLS * cgrid + 8 * q[irep] + 2 * w_loc[irep]) * SLOTS
        + qoff.ravel()
    )
    nz = np.nonzero(flat_deg)[0]
    nz_deg = flat_deg[nz]
    reps = np.repeat(np.arange(len(nz)), nz_deg)
    csum = np.zeros(len(nz) + 1, np.int64)
    csum[1:] = np.cumsum(nz_deg)
    within = np.arange(int(nz_deg.sum()), dtype=np.int64) - csum[reps]
    e_pos = e_start[nz][reps] + within
    slot_sv = sv_base[nz][reps] + within
    slot_ix = ix_base[nz][reps] + within

    idx_slot = np.zeros(G * SLOTS, np.int32)
    val_slot = np.zeros(G * SLOTS, np.float32)
    seg_slot = np.zeros(G * SLOTS, np.int16)
    idx_slot[slot_ix] = cols_s[e_pos] % CHUNK
    val_slot[slot_sv] = vals_s[e_pos]
    seg_slot[slot_sv] = items_rank[irep[nz]][reps]

    vrow = (items_w // WPC) * VPC + (items_w % WPC) * W_RANK + items_rank
    return dict(
        n_windows=n_windows,
        G=G,
        idx=idx_slot,
        val=val_slot,
        seg=seg_slot,
        vrow=vrow,
        dest=items_dest,
        n_edges=len(rows_loc),
    )


def pack_all(edge_row, edge_col, edge_val, n_nodes=N_NODES, n_cores=N_CORES):
    npc = n_nodes // n_cores
    core_id = edge_row // npc
    packs = []
    for i in range(n_cores):
        m = core_id == i
        packs.append(
            pack_core(edge_row[m] - i * npc, edge_col[m], edge_val[m], npc)
        )
    return packs


def build_call_arrays(p, n_calls):
    """DRAM layouts, one row per PAIR of F_CALLS calls:

      idx   [n_pairs, 32, NCHUNK*iw] int16 — 4 chunk-gather index blocks,
            16-wrapped, replicated across partitions 0..31 (all the queue-0
            gather ucode reads); the device zeroes partitions 32..127 of the
            SBUF tile.
      segval[n_quads, 128, 4*F_CALLS*2*CG] int16 — per-call [seg | val]
            blocks (bf16 bit-packed), one element per slot partition,
            packed four pairs per row so the per-partition DMA descriptor
            reaches 512B (below that the cost doubles per byte).

    The dma_gather for (pair, chunk c) consumes the pair's 8*F_CALLS chunk-c
    groups in order; index position i -> (partition i%128, group of the pair
    tile), wrapped so position i sits at [i%16, i//16].
    """
    G = p["G"]
    gtot = n_calls * CG
    n_pairs = n_calls // F_CALLS

    def lay(a, np_dtype):
        full = np.zeros(gtot * SLOTS, a.dtype)
        full[: G * SLOTS] = a
        return np.ascontiguousarray(
            full.reshape(n_calls, CG, SLOTS).transpose(0, 2, 1)
        ).astype(np_dtype)

    idx_full = np.zeros(gtot * SLOTS, np.int64)
    idx_full[: G * SLOTS] = p["idx"]
    byg = idx_full.reshape(n_pairs, F_CALLS * CG, SLOTS)
    iw = GATHER_IDX // 16
    idx_arr = np.empty((n_pairs, 32, NCHUNK * iw), np.int16)
    gpc = 4 * F_CALLS * 2  # groups per chunk per gather tile
    for c in range(NCHUNK):
        flat = byg[:, gpc * c : gpc * (c + 1), :].reshape(n_pairs, GATHER_IDX)
        wrapped = flat.reshape(n_pairs, iw, 16).transpose(0, 2, 1)
        idx_arr[:, :, c * iw : (c + 1) * iw] = np.tile(wrapped, (1, 2, 1)).astype(
            np.int16
        )
    svw = F_CALLS * 2 * CG
    n_quads = (n_pairs + 3) // 4
    segval = np.zeros((n_quads * 4, 128, svw), np.int16)
    seg_l = lay(p["seg"], BF16).view(np.int16)  # [n_calls, 128, CG]
    val_l = lay(p["val"], BF16).view(np.int16)
    for qq in range(F_CALLS):
        base = qq * 2 * CG
        segval[:n_pairs, :, base : base + CG] = seg_l[qq::F_CALLS]
        segval[:n_pairs, :, base + CG : base + 2 * CG] = val_l[qq::F_CALLS]
    segval = np.ascontiguousarray(
        segval.reshape(n_quads, 4, 128, svw).transpose(0, 2, 1, 3).reshape(
            n_quads, 128, 4 * svw
        )
    )
    return np.ascontiguousarray(idx_arr), segval


# ----------------------------------------------------------------------------
# Device program
# ----------------------------------------------------------------------------

def build_program(n_calls, n_nodes=N_NODES, d=D, chunk=CHUNK):
    nc = bacc.Bacc("TRN2", target_bir_lowering=False, debug=False)
    f32 = mybir.dt.float32
    bf16 = mybir.dt.bfloat16

    x = nc.dram_tensor("xb", [n_nodes, d], bf16, kind="ExternalInput")
    n_pairs = n_calls // F_CALLS
    iw = GATHER_IDX // 16  # idx words per chunk-gather per partition
    iww = NCHUNK * iw  # idx words per partition per pair
    svw = F_CALLS * 2 * CG  # seg/val words per partition per pair
    idxT = nc.dram_tensor(
        "idx", [n_pairs, 32, iww], mybir.dt.int16, kind="ExternalInput"
    )
    n_quads = (n_pairs + 3) // 4
    svT = nc.dram_tensor(
        "segval", [n_quads, 128, 4 * svw], mybir.dt.int16, kind="ExternalInput"
    )
    wtT = nc.dram_tensor("wt", [d // 128, 128, d], bf16, kind="ExternalInput")
    iotaT = nc.dram_tensor("iota32", [128, W_RANK], bf16, kind="ExternalInput")
    # outT[cl, p, h*VPC + j] = outT_feat[h*128 + p, cl*VPC + j]
    out = nc.dram_tensor(
        "out", [n_calls, 128, (d // 128) * VPC], bf16, kind="ExternalOutput"
    )

    kh = d // 128  # feature half-tiles
    n_chunks = (n_nodes + chunk - 1) // chunk
    fcg = F_CALLS * CG  # groups per pair tile
    gpc = fcg // NCHUNK  # groups per chunk within a pair tile

    with tile.TileContext(nc) as tc, ExitStack() as ctx:
        const = ctx.enter_context(tc.tile_pool(name="const", bufs=1))
        sb = ctx.enter_context(tc.tile_pool(name="sb", bufs=4))
        svp = ctx.enter_context(tc.tile_pool(name="svp", bufs=6))
        xgp = ctx.enter_context(tc.tile_pool(name="xg", bufs=3))
        ps = ctx.enter_context(tc.tile_pool(name="ps", bufs=2, space="PSUM"))

        nc.gpsimd.load_library(_mlp_lib)

        # idx loads ride the SP HWDGE queue alone (so a seg/val buffer wait
        # can never head-of-line-block the next gather's indices); seg/val
        # loads ride the Activation queue AHEAD of the out stores, prefetched
        # one pair deep. The first 4 idx loads cover all 128 partitions to
        # initialize the 4 cycling pool buffers; later loads touch only
        # partitions 0..31 (all the queue-0 gather ucode reads).
        def load_idx(pr):
            t = sb.tile([128, iww], mybir.dt.int16, tag="idx", name=f"idx{pr}")
            nc.sync.dma_start(t[0:32, :], idxT[pr])
            # partitions 32..127 are never read by the queue-0 gather ucode;
            # zero them so the 128-partition idxs_ap reads defined memory
            # (32-partition pieces: engine APs must start at 0/32/64/96)
            for pb in (32, 64, 96):
                nc.vector.memset(t[pb : pb + 32, :], 0)
            return t

        def load_sv(qd):
            t = svp.tile(
                [128, 4 * svw], mybir.dt.int16, tag="sv", name=f"sv{qd}"
            )
            nc.scalar.dma_start(t[:], svT[qd])
            return t

        # pair 0's meta loads first so the first gather issues ASAP; the
        # (small) const loads fill DMA slack during the first gather.
        # idx is prefetched 2 pairs deep; sv quads (4 pairs each) one quad
        # deep, so an out store holding the Activation SEQ can't delay a
        # seg/val load into its consumer.
        D_IDX = 2
        idx_tiles = {pr: load_idx(pr) for pr in range(min(D_IDX, n_pairs))}
        sv_quads = {0: load_sv(0)}

        wt_t = const.tile([128, kh * d], bf16)
        for h in range(kh):
            nc.sync.dma_start(wt_t[:, h * d : (h + 1) * d], wtT[h])
        iota_t = const.tile([128, W_RANK], bf16)
        nc.sync.dma_start(iota_t[:], iotaT[:, :])

        for pr in range(n_pairs):
            if pr + D_IDX < n_pairs:
                idx_tiles[pr + D_IDX] = load_idx(pr + D_IDX)
            if pr % 4 == 0 and pr // 4 + 1 < n_quads:
                sv_quads[pr // 4 + 1] = load_sv(pr // 4 + 1)
            idx_t = idx_tiles.pop(pr)
            sv_t = sv_quads[pr // 4]
            svo = (pr % 4) * svw
            if pr % 4 == 3 or pr == n_pairs - 1:
                sv_quads.pop(pr // 4)

            xg = xgp.tile([SLOTS, fcg, d], bf16, tag="xg")
            for c in range(min(n_chunks, NCHUNK)):
                lo = c * chunk
                hi = min(n_nodes, lo + chunk)
                nc.gpsimd.dma_gather(
                    xg[:, gpc * c : gpc * (c + 1), :],
                    x[lo:hi, :],
                    idx_t[:, c * iw : (c + 1) * iw],
                    GATHER_IDX,
                    GATHER_IDX,
                    d,
                )

            for q in range(F_CALLS):
                cl = pr * F_CALLS + q
                o = svo + q * 2 * CG
                seg_t = sv_t[:, o : o + CG].bitcast(bf16)
                val_t = sv_t[:, o + CG : o + 2 * CG].bitcast(bf16)

                # banded scaled one-hot: S[p, g, r] = val[p,g] * (seg[p,g] == r)
                d1 = sb.tile([SLOTS, CG, W_RANK], bf16, tag="d1")
                nc.vector.tensor_tensor(
                    out=d1[:],
                    in0=seg_t.unsqueeze(2).to_broadcast([SLOTS, CG, W_RANK]),
                    in1=iota_t[:].unsqueeze(1).to_broadcast([SLOTS, CG, W_RANK]),
                    op=mybir.AluOpType.subtract,
                )
                s_t = sb.tile([SLOTS, CG, W_RANK], bf16, tag="s")
                nc.vector.scalar_tensor_tensor(
                    out=s_t[:],
                    in0=d1[:],
                    scalar=0.0,
                    op0=mybir.AluOpType.is_equal,
                    in1=val_t.unsqueeze(2).to_broadcast([SLOTS, CG, W_RANK]),
                    op1=mybir.AluOpType.mult,
                )

                # transposed aggregation: paccT_h[feat 128, VPC call-dests],
                # window w_loc writes columns [w_loc*W_RANK, (w_loc+1)*W_RANK)
                paccT = []
                for h in range(kh):
                    pt = ps.tile([128, VPC], f32, tag=f"paccT{h}", name=f"paccT{h}")
                    paccT.append(pt)
                for w_loc in range(WPC):
                    off = w_loc * W_RANK
                    for h in range(kh):
                        for c in range(NCHUNK):
                            for j in range(2):
                                gq = 8 * c + 2 * w_loc + j  # per-call group
                                gt = (
                                    gpc * c + 8 * q + 2 * w_loc + j
                                )  # pair-tile group
                                nc.tensor.matmul(
                                    out=paccT[h][:, off : off + W_RANK],
                                    lhsT=xg[:, gt, h * 128 : (h + 1) * 128],
                                    rhs=s_t[:, gq, :],
                                    start=(c == 0 and j == 0),
                                    stop=(c == NCHUNK - 1 and j == 1),
                                )

                # aggT to SBUF bf16, then W projection (still transposed):
                # outT[ho*128+p, dest] = sum_fi W.T[fi, ho*128+p] aggT[fi, dest]
                # copy pairs split across DVE and the (otherwise idle)
                # Activation engine so the two halves land in parallel
                aggT = sb.tile([128, kh, VPC], bf16, tag="aggT")
                nc.vector.tensor_copy(out=aggT[:, 0, :], in_=paccT[0][:])
                nc.scalar.copy(out=aggT[:, 1, :], in_=paccT[1][:])
                po = []
                for h in range(kh):
                    pt = ps.tile([128, VPC], f32, tag=f"po{h}", name=f"po{h}")
                    po.append(pt)
                for ho in range(kh):
                    for hi in range(kh):
                        nc.tensor.matmul(
                            out=po[ho][:],
                            lhsT=wt_t[:, hi * d + ho * 128 : hi * d + (ho + 1) * 128],
                            rhs=aggT[:, hi, :],
                            start=(hi == 0),
                            stop=(hi == kh - 1),
                        )
                osb = sb.tile([128, kh * VPC], bf16, tag="osb")
                nc.vector.tensor_copy(out=osb[:, 0:VPC], in_=po[0][:])
                nc.scalar.copy(out=osb[:, VPC : 2 * VPC], in_=po[1][:])
                nc.scalar.dma_start(out[cl], osb[:])

    nc.compile()
    return nc


# ----------------------------------------------------------------------------
# Entry point
# ----------------------------------------------------------------------------

_PROG_CACHE = {}


def _get_program(n_calls):
    if n_calls not in _PROG_CACHE:
        _PROG_CACHE[n_calls] = build_program(n_calls)
    return _PROG_CACHE[n_calls]


def make_in_maps(x, W, packs, n_calls):
    xb = np.ascontiguousarray(x.astype(BF16))
    wt = np.ascontiguousarray(W.T.reshape(D // 128, 128, D).astype(BF16))
    iota = np.broadcast_to(np.arange(W_RANK, dtype=np.float32), (128, W_RANK))
    iota = np.ascontiguousarray(iota.astype(BF16))
    in_maps = []
    for p in packs:
        idx_arr, segval = build_call_arrays(p, n_calls)
        in_maps.append(
            dict(xb=xb, idx=idx_arr, segval=segval, wt=wt, iota32=iota)
        )
    return in_maps


def kernel(x, W, edge_val, edge_row, edge_col, _return_results=False, trace=False):
    packs = pack_all(edge_row, edge_col, edge_val)
    n_calls = max(p["G"] // CG for p in packs)
    n_calls = ((n_calls + F_CALLS - 1) // F_CALLS) * F_CALLS
    nc = _get_program(n_calls)
    in_maps = make_in_maps(x, W, packs, n_calls)
    res = run_bass_kernel_spmd(
        nc, in_maps, core_ids=list(range(N_CORES)), trace=trace
    )
    out = np.zeros((N_NODES, D), np.float32)
    for i, (p, core_out) in enumerate(zip(packs, res.results)):
        # outT[cl, p, h*VPC+j] -> virtual-dest-major [n_calls*VPC, 256]
        ot = np.asarray(core_out["out"]).astype(np.float32)
        ot = ot.reshape(n_calls, 128, D // 128, VPC)
        # feature f = h*128 + p ; virtual dest v = cl*VPC + j
        ov = ot.transpose(0, 3, 2, 1).reshape(n_calls * VPC, D)
        true_ids = p["dest"] + i * NPC
        if len(np.unique(true_ids)) == len(true_ids):
            out[true_ids] = ov[p["vrow"]]
        else:
            np.add.at(out, true_ids, ov[p["vrow"]])
    if _return_results:
        return out, res
    return out
